# revision 1
# baseline (speedup 1.0000x reference)
"""Trainium2 Bass kernel for nn_MetricLoss (segment_reduce / discriminative loss).

Reference math (K=32 labels, D=16):
  cents[s,k,:]  = mean of embeddings of sample s where label==k
  push[s]       = sum_{k<j} relu(0.25 - L1(c_sk, c_sj))^2 / 496
  pull[s]       = mean over ALL B*H*W pixels p of  L1(e_p, c_s,label_p)^2
  loss          = mean_s (push[s] + 0.1 * pull[s])

Strategy (8 cores, two launches, pixel-major layout [128 part x 576 cols]):
  Launch A: per-core partial centroid sums+counts.
    - one-hot(labels) in bf16 via DVE is_equal
    - PE: 84 groups; weights = 7 pixel-tiles of [emb(16) ; ones(1)] = 119 cols
      (padded to 128), rhs = the 7 tiles' one-hot [128, 224]; accumulated into
      a single PSUM bank; diagonal blocks hold per-tile-class sums+counts.
    - host sums blocks across groups/cores -> cents [4,32,16]
  Launch B: pull + push.
    - onehotT4 [128=(strip4,k32), 18432] via DVE tensor_scalar is_equal (4x)
    - gather: per pixel-tile MM with lhsT = onehotT4 strip slice [32,128],
      rhs = centsT [32,64] -> psum [128 px, 64=(b,d)] = gathered centroids
    - DVE: diff = psum - emb (b-broadcast), |.|-reduce over d -> dist [128,4,576]
    - DVE: per-b sum of dist^2 -> pacc [128,4] -> host reduce
    - push computed redundantly per core from tiny cent tables.
"""

import numpy as np
import ml_dtypes

import concourse.bass as bass
import concourse.bacc as bacc
import concourse.mybir as mybir
from concourse.tile import TileContext
from concourse.bass_utils import run_bass_kernel_spmd

BF16 = ml_dtypes.bfloat16
F32 = np.float32

# problem constants (hardcoded per contract)
B, H, W, D, K = 4, 384, 384, 16, 32
NCORES = 8
NPIX_TOT = B * H * W              # 589824
NPIX = NPIX_TOT // NCORES         # 73728 per core
P = 128                           # partitions
TC = NPIX // P                    # 576 pixel columns per partition
TCP = 588                         # padded to 7*84 for launch A grouping
NG = TCP // 7                     # 84 weight groups
GW = 7 * 17                       # 119 weight cols per group
WCOLS = GW * (NG - 1) + 128       # 10005 -> pad
WCOLS_PAD = 10016
NB = TC // 8                      # 72 gather batches (8 tiles each)
QS = TC // 4                      # 144 tiles per strip
LAB_PAD = 100.0                   # pad label (!= any of 0..31)

PUSH_MARGIN = 0.25
PUSH_W = 1.0
PULL_W = 0.1
NCMP = K * (K - 1) / 2.0

_built = {}


def _build_launch_a():
    nc = bacc.Bacc("TRN2", target_bir_lowering=False, debug=False)
    bf = mybir.dt.bfloat16
    f32 = mybir.dt.float32

    emb17 = nc.dram_tensor("emb17", [P, WCOLS_PAD], bf, kind="ExternalInput")
    labels = nc.dram_tensor("labels", [P, TCP], bf, kind="ExternalInput")
    outA = nc.dram_tensor("outA", [P, 224], f32, kind="ExternalOutput")

    with TileContext(nc) as tc:
        with (
            tc.tile_pool(name="sbuf", bufs=1) as pool,
            tc.tile_pool(name="psum", bufs=1, space="PSUM") as psum_pool,
        ):
            emb_sb = pool.tile([P, WCOLS_PAD], bf)
            lab_sb = pool.tile([P, TCP], bf)
            onehot = pool.tile([P, K, TCP], bf)  # k-major: 4x-mode generation

            nc.sync.dma_start(out=lab_sb[:], in_=labels.ap())
            # emb17 in 4 chunks for DMA/PE overlap
            NCH = 4
            ch = WCOLS_PAD // NCH
            for i in range(NCH):
                nc.sync.dma_start(
                    out=emb_sb[:, i * ch : (i + 1) * ch],
                    in_=emb17.ap()[:, i * ch : (i + 1) * ch],
                )

            # one-hot: per-k tensor_scalar is_equal (single-src bf16 -> 4x mode)
            half = TCP // 2
            for h in range(2):
                sl = slice(h * half, (h + 1) * half)
                for k in range(K):
                    nc.vector.tensor_scalar(
                        out=onehot[:, k, sl],
                        in0=lab_sb[:, sl],
                        scalar1=float(k),
                        scalar2=None,
                        op0=mybir.AluOpType.is_equal,
                    )

            ps = psum_pool.tile([P, 7, K], mybir.dt.float32)
            for g in range(NG):
                nc.tensor.matmul(
                    ps[:],
                    emb_sb[:, GW * g : GW * g + 128],
                    onehot[:, :, 7 * g : 7 * g + 7].rearrange("p k t -> p t k"),
                    start=(g == 0),
                    stop=(g == NG - 1),
                )

            evac = pool.tile([P, 7 * K], f32)
            nc.vector.tensor_copy(out=evac[:], in_=ps[:].rearrange("p a b -> p (a b)"))
            nc.sync.dma_start(out=outA.ap(), in_=evac[:])
    nc.compile()
    return nc


def _build_launch_b():
    nc = bacc.Bacc("TRN2", target_bir_lowering=False, debug=False)
    bf = mybir.dt.bfloat16
    f32 = mybir.dt.float32

    emb16 = nc.dram_tensor("emb16", [P, TC * D], bf, kind="ExternalInput")
    lab4 = nc.dram_tensor("lab4", [P, QS * P], bf, kind="ExternalInput")
    iotaT = nc.dram_tensor("iotaT", [P, 1], f32, kind="ExternalInput")
    centsT = nc.dram_tensor("centsT", [P, 64], bf, kind="ExternalInput")
    cpp = nc.dram_tensor("cpp", [P, D], bf, kind="ExternalInput")
    cjd = nc.dram_tensor("cjd", [P, K * D], bf, kind="ExternalInput")
    triu = nc.dram_tensor("triu", [P, K], bf, kind="ExternalInput")
    pacc_d = nc.dram_tensor("pacc", [P, 4], f32, kind="ExternalOutput")
    pushp_d = nc.dram_tensor("pushp", [P, 1], f32, kind="ExternalOutput")

    with TileContext(nc) as tc:
        with (
            tc.tile_pool(name="sbuf", bufs=1) as pool,
            tc.tile_pool(name="work", bufs=3) as wpool,
            tc.tile_pool(name="psum", bufs=2, space="PSUM") as psum_pool,
        ):
            emb_sb = pool.tile([P, TC, D], bf)
            lab4_sb = pool.tile([P, QS * P], bf)
            iotaT_sb = pool.tile([P, 1], f32)
            centsT_sb = pool.tile([P, 64], bf)
            cpp_sb = pool.tile([P, D], bf)
            cjd_sb = pool.tile([P, K, D], bf)
            triu_sb = pool.tile([P, K], bf)
            oh4 = pool.tile([P, QS * P], bf)
            dist = pool.tile([P, TC, 4], bf)  # t-major, b-inner (2x-mode APs)
            pacc = pool.tile([P, 4], f32)
            pushp = pool.tile([P, 1], f32)

            nc.sync.dma_start(out=iotaT_sb[:], in_=iotaT.ap())
            nc.sync.dma_start(out=centsT_sb[:], in_=centsT.ap())
            nc.sync.dma_start(out=cpp_sb[:], in_=cpp.ap())
            nc.sync.dma_start(out=cjd_sb[:], in_=cjd.ap().rearrange("p (a b) -> p a b", b=D))
            nc.sync.dma_start(out=triu_sb[:], in_=triu.ap())

            NCH = 4
            ech = TC // NCH
            for i in range(NCH):
                nc.sync.dma_start(
                    out=emb_sb[:, i * ech : (i + 1) * ech, :],
                    in_=emb16.ap().rearrange("p (t d) -> p t d", d=D)[
                        :, i * ech : (i + 1) * ech, :
                    ],
                )
            lch = (QS * P) // NCH
            for i in range(NCH):
                nc.sync.dma_start(
                    out=lab4_sb[:, i * lch : (i + 1) * lch],
                    in_=lab4.ap()[:, i * lch : (i + 1) * lch],
                )

            # one-hot (transposed, 4 strips) via tensor_scalar is_equal (4x mode)
            NOH = 12
            oc = (QS * P) // NOH
            for i in range(NOH):
                sl = slice(i * oc, (i + 1) * oc)
                nc.vector.tensor_scalar(
                    out=oh4[:, sl],
                    in0=lab4_sb[:, sl],
                    scalar1=iotaT_sb[:, 0:1],
                    scalar2=None,
                    op0=mybir.AluOpType.is_equal,
                )

            # gather + pull distance; superbatches of 32 tiles, one PSUM bank
            # per strip (concurrent row-strip MMs must hit distinct banks).
            # MM emission interleaves strips so next LDW overlaps current MM.
            NSB = TC // 32
            for sb in range(NSB):
                t0 = 32 * sb
                pss = [
                    psum_pool.tile(
                        [P, 8, 4, D], mybir.dt.float32, tag=f"ps{s}",
                        name=f"ps{s}_{sb}",
                    )
                    for s in range(4)
                ]
                for j in range(8):
                    for s in range(4):
                        q = 8 * sb + j
                        nc.tensor.matmul(
                            pss[s][:, j, :, :].rearrange("p a b -> p (a b)"),
                            oh4[32 * s : 32 * s + 32, P * q : P * (q + 1)],
                            centsT_sb[32 * s : 32 * s + 32, :],
                            start=True,
                            stop=True,
                            tile_position=(32 * s, 0),
                        )
                for s in range(4):
                    gev = wpool.tile([P, 8, 4, D], bf, tag=f"gev{s}")
                    nc.scalar.copy(out=gev[:], in_=pss[s][:])
                    diff = wpool.tile([P, 8, 4, D], bf, tag=f"diff{s}")
                    nc.vector.tensor_tensor(
                        out=diff[:],
                        in0=gev[:],
                        in1=emb_sb[:, t0 + s : t0 + s + 29 : 4, :]
                        .unsqueeze(2)
                        .broadcast_to([P, 8, 4, D]),
                        op=mybir.AluOpType.subtract,
                    )
                    with nc.allow_low_precision("dist in bf16; error averages out"):
                        nc.vector.tensor_reduce(
                            out=dist[:, t0 + s : t0 + s + 29 : 4, :],
                            in_=diff[:],
                            axis=mybir.AxisListType.X,
                            op=mybir.AluOpType.add,
                            apply_absolute_value=True,
                        )

            # pull partial: pacc[p, b] = sum_t dist^2
            sq = pool.tile([P, TC, 4], f32)
            nc.vector.tensor_tensor(
                out=sq[:], in0=dist[:], in1=dist[:], op=mybir.AluOpType.mult
            )
            nc.vector.tensor_reduce(
                out=pacc[:],
                in_=sq[:].rearrange("p t b -> p b t"),
                axis=mybir.AxisListType.X,
                op=mybir.AluOpType.add,
            )
            nc.sync.dma_start(out=pacc_d.ap(), in_=pacc[:])

            # push (tiny, redundant per core): partitions p=(b,k)
            pd_diff = pool.tile([P, K, D], bf)
            nc.vector.tensor_tensor(
                out=pd_diff[:],
                in0=cpp_sb[:].unsqueeze(1).broadcast_to([P, K, D]),
                in1=cjd_sb[:],
                op=mybir.AluOpType.subtract,
            )
            pd = pool.tile([P, K], f32)
            nc.vector.tensor_reduce(
                out=pd[:],
                in_=pd_diff[:],
                axis=mybir.AxisListType.X,
                op=mybir.AluOpType.add,
                apply_absolute_value=True,
            )
            # relu(margin - d)^2 == min(d - margin, 0)^2
            m = pool.tile([P, K], f32)
            nc.vector.tensor_scalar(
                out=m[:],
                in0=pd[:],
                scalar1=PUSH_MARGIN,
                scalar2=0.0,
                op0=mybir.AluOpType.subtract,
                op1=mybir.AluOpType.min,
            )
            msq = pool.tile([P, K], f32)
            nc.vector.tensor_tensor(
                out=msq[:], in0=m[:], in1=m[:], op=mybir.AluOpType.mult
            )
            msqm = pool.tile([P, K], f32)
            nc.vector.tensor_tensor(
                out=msqm[:], in0=msq[:], in1=triu_sb[:], op=mybir.AluOpType.mult
            )
            nc.vector.tensor_reduce(
                out=pushp[:],
                in_=msqm[:],
                axis=mybir.AxisListType.X,
                op=mybir.AluOpType.add,
            )
            nc.sync.dma_start(out=pushp_d.ap(), in_=pushp[:])
    nc.compile()
    return nc


def _get(name):
    if name not in _built:
        if name == "A":
            _built[name] = _build_launch_a()
        else:
            _built[name] = _build_launch_b()
    return _built[name]


def _prep_a(emb_flat, lab_flat):
    """emb_flat [NPIX_TOT, D] f32, lab_flat [NPIX_TOT] i32 -> per-core in_maps."""
    in_maps = []
    for c in range(NCORES):
        e = emb_flat[c * NPIX : (c + 1) * NPIX].astype(BF16).reshape(P, TC, D)
        l = lab_flat[c * NPIX : (c + 1) * NPIX].reshape(P, TC)
        e17 = np.zeros((P, TCP, 17), dtype=BF16)
        e17[:, :TC, :D] = e
        e17[:, :, D] = BF16(1.0)
        w = np.zeros((P, WCOLS_PAD), dtype=BF16)
        w[:, : TCP * 17] = e17.reshape(P, TCP * 17)
        lb = np.full((P, TCP), LAB_PAD, dtype=BF16)
        lb[:, :TC] = l.astype(BF16)
        in_maps.append({"emb17": w, "labels": lb})
    return in_maps


def _reduce_a(results):
    """outA [8][P, 224] -> cents [B, K, D] float64, counts [B, K]."""
    sums = np.zeros((B, K, D), dtype=np.float64)
    cnts = np.zeros((B, K), dtype=np.float64)
    for c in range(NCORES):
        o = results[c]["outA"].astype(np.float64).reshape(P, 7, K)
        s = c // 2
        for j in range(7):
            blk = o[17 * j : 17 * j + 17, j, :]  # [17, K]
            sums[s] += blk[:D].T  # [K, D]
            cnts[s] += blk[D]
    cents = sums / np.maximum(cnts, 1.0)[:, :, None]
    cents = np.where(cnts[:, :, None] > 0, cents, 0.0)
    return cents, cnts


def _prep_b(emb_flat, lab_flat, cents):
    iotaT = (np.arange(P, dtype=F32) % K).astype(F32).reshape(P, 1)
    centsT = np.zeros((P, 64), dtype=BF16)
    cb = cents.astype(F32)  # [B, K, D]
    for s in range(4):
        # centsT[32s+k, 16b+d] = cents[b, k, d]
        centsT[32 * s : 32 * s + 32, :] = (
            cb.transpose(1, 0, 2).reshape(K, 64).astype(BF16)
        )
    cpp = cb.reshape(P, D).astype(BF16)  # p = 32b + k
    cjd = np.zeros((P, K * D), dtype=BF16)
    for b in range(4):
        cjd[32 * b : 32 * b + 32, :] = np.broadcast_to(
            cb[b].reshape(1, K * D), (K, K * D)
        ).astype(BF16)
    triu = np.zeros((P, K), dtype=BF16)
    kk = np.arange(K)
    for b in range(4):
        triu[32 * b : 32 * b + 32, :] = (kk[None, :] > kk[:, None]).astype(BF16)

    in_maps = []
    for c in range(NCORES):
        e = emb_flat[c * NPIX : (c + 1) * NPIX].astype(BF16).reshape(P, TC, D)
        l = lab_flat[c * NPIX : (c + 1) * NPIX].reshape(P, TC)  # [m, tau]
        lab4 = np.zeros((P, QS * P), dtype=BF16)
        for s in range(4):
            a = l[:, s::4]  # [m, q]
            lab4[32 * s : 32 * s + 32, :] = np.broadcast_to(
                a.T.reshape(1, QS * P), (K, QS * P)
            ).astype(BF16)
        in_maps.append(
            {
                "emb16": e.reshape(P, TC * D),
                "lab4": lab4,
                "iotaT": iotaT.copy(),
                "centsT": centsT.copy(),
                "cpp": cpp.copy(),
                "cjd": cjd.copy(),
                "triu": triu.copy(),
            }
        )
    return in_maps


def run_launches(embeddings, labels, trace=False, trace_kwargs=None):
    """Returns (loss_scalar, resA, resB) — resA/resB are BassKernelResults."""
    emb_flat = np.ascontiguousarray(np.asarray(embeddings), dtype=F32).reshape(
        NPIX_TOT, D
    )
    lab_flat = np.ascontiguousarray(np.asarray(labels), dtype=np.int32).reshape(
        NPIX_TOT
    )
    core_ids = list(range(NCORES))

    kwA = dict(trace=trace, **(trace_kwargs or {}))
    resA = run_bass_kernel_spmd(_get("A"), _prep_a(emb_flat, lab_flat), core_ids, **kwA)
    cents, _ = _reduce_a(resA.results)

    resB = run_bass_kernel_spmd(
        _get("B"), _prep_b(emb_flat, lab_flat, cents), core_ids, **kwA
    )
    pull = np.zeros(4, dtype=np.float64)
    for c in range(NCORES):
        pull += resB.results[c]["pacc"].astype(np.float64).sum(axis=0)
    pull /= NPIX_TOT

    pushp = resB.results[0]["pushp"].astype(np.float64).reshape(4, K).sum(axis=1)
    push = pushp / NCMP

    loss = np.mean(PUSH_W * push + PULL_W * pull)
    return np.array(loss, dtype=F32), resA, resB


def kernel(embeddings, labels):
    loss, _, _ = run_launches(embeddings, labels, trace=False)
    return loss



# revision 4
# speedup vs baseline: 1.2692x; 1.2692x over previous
"""Trainium2 Bass kernel for nn_MetricLoss (segment_reduce / discriminative loss).

Reference math (K=32 labels, D=16):
  cents[s,k,:]  = mean of embeddings of sample s where label==k
  push[s]       = sum_{k<j} relu(0.25 - L1(c_sk, c_sj))^2 / 496
  pull[s]       = mean over ALL B*H*W pixels p of  L1(e_p, c_s,label_p)^2
  loss          = mean_s (push[s] + 0.1 * pull[s])

Strategy (8 cores, 2 launches, SORT-BASED):
  Host sorts each core's 73728 pixels by label into a balanced layout
  [128 partitions, S slots, 16], where label k owns a uniform column
  range of C_k slots on every partition (C_k = max over cores of
  ceil(count/128)); unused slots are zero-padded.  Pixel order is
  irrelevant because both the centroid sums and the pull term are plain
  sums over pixels.

  Launch A: per-core centroid partial sums = per-label TensorReduce
    over the slot axis -> partials [128, 32*16] f32.  Host reduces
    partitions + core pairs, divides by exact counts -> cents.
    (Counts are known on host from the sort; push is computed on host.)

  Launch B: pull.  ctab [128, 32,4,16] = bf16 cents replicated per
    partition.  Per label: diff = emb - c broadcast (DVE TT, 2x mode);
    |.| in-place on ACT (Abs activation); Manhattan sum over d via a
    log2 tree of TT adds (DVE 2x / POOL split); dist^2 summed over
    slots via tensor_tensor_reduce -> pacc [128, 4].  Host subtracts
    the zero-pad contribution n_pad_k * L1(c_bk)^2 and normalizes.
"""

import numpy as np
import ml_dtypes

import concourse.bass as bass
import concourse.bacc as bacc
import concourse.mybir as mybir
from concourse.tile import TileContext
from concourse.bass_utils import run_bass_kernel_spmd

BF16 = ml_dtypes.bfloat16
F32 = np.float32

B, H, W, D, K = 4, 384, 384, 16, 32
NCORES = 8
NPIX_TOT = B * H * W              # 589824
NPIX = NPIX_TOT // NCORES         # 73728 per core
P = 128

PUSH_MARGIN = 0.25
PUSH_W = 1.0
PULL_W = 0.1
NCMP = K * (K - 1) / 2.0

_built = {}


# ---------------------------------------------------------------- layout

def _layout(lab_flat):
    """lab_flat [NPIX_TOT] int32 -> dict describing the sorted layout.

    C[k]: slots per partition for label k (uniform across cores);
    off[k]: slot offset; S: total slots; counts[core, k]; per-core
    pixmap [P, S] int64 (global pixel index, or -1 for pad).
    """
    counts = np.zeros((NCORES, K), dtype=np.int64)
    idx_by = []
    for c in range(NCORES):
        lab = lab_flat[c * NPIX : (c + 1) * NPIX]
        counts[c] = np.bincount(lab, minlength=K)
        order = np.argsort(lab, kind="stable")
        idx_by.append(np.split(order, np.cumsum(counts[c])[:-1]))
    C = np.maximum(1, (counts.max(axis=0) + P - 1) // P).astype(np.int64)
    off = np.concatenate([[0], np.cumsum(C)])
    S = int(off[-1])
    pixmaps = []
    for c in range(NCORES):
        pm = np.full((P, S), -1, dtype=np.int64)
        for k in range(K):
            ck = int(counts[c, k])
            pad = np.full(int(C[k]) * P, -1, dtype=np.int64)
            pad[:ck] = idx_by[c][k] + c * NPIX
            pm[:, off[k] : off[k + 1]] = pad.reshape(int(C[k]), P).T
        pixmaps.append(pm)
    return {"C": C, "off": off, "S": S, "counts": counts, "pixmaps": pixmaps}


def _emb_sorted(emb_flat, lay):
    """emb_flat [NPIX_TOT, D] f32 -> per-core [P, S*D] bf16 (pad=0)."""
    S = lay["S"]
    emb_pad = np.vstack([emb_flat, np.zeros((1, D), dtype=emb_flat.dtype)])
    outs = []
    for pm in lay["pixmaps"]:
        g = emb_pad[np.where(pm < 0, NPIX_TOT, pm)]  # [P, S, D]
        outs.append(np.ascontiguousarray(g.astype(BF16).reshape(P, S * D)))
    return outs


# ---------------------------------------------------------------- launch A

def _build_a(C, S):
    nc = bacc.Bacc("TRN2", target_bir_lowering=False, debug=False)
    bf = mybir.dt.bfloat16
    f32 = mybir.dt.float32

    emb_d = nc.dram_tensor("emb", [P, S * D], bf, kind="ExternalInput")
    part_d = nc.dram_tensor("part", [P, K * D], f32, kind="ExternalOutput")

    off = np.concatenate([[0], np.cumsum(C)])
    with TileContext(nc) as tc:
        with tc.tile_pool(name="sbuf", bufs=1) as pool:
            emb = pool.tile([P, S, D], bf)
            partials = pool.tile([P, K, D], f32)

            NCH = 4
            bnd = [S * i // NCH for i in range(NCH + 1)]
            for i in range(NCH):
                nc.sync.dma_start(
                    out=emb[:, bnd[i] : bnd[i + 1], :],
                    in_=emb_d.ap().rearrange("p (t d) -> p t d", d=D)[
                        :, bnd[i] : bnd[i + 1], :
                    ],
                )
            for k in range(K):
                nc.vector.tensor_reduce(
                    out=partials[:, k, :],
                    in_=emb[:, off[k] : off[k + 1], :].rearrange("p t d -> p d t"),
                    axis=mybir.AxisListType.X,
                    op=mybir.AluOpType.add,
                )
            nc.sync.dma_start(
                out=part_d.ap(), in_=partials[:].rearrange("p a b -> p (a b)")
            )
    nc.compile()
    return nc


# ---------------------------------------------------------------- launch B

def _build_b(C, S):
    nc = bacc.Bacc("TRN2", target_bir_lowering=False, debug=False)
    bf = mybir.dt.bfloat16
    f32 = mybir.dt.float32
    S4 = S * 4

    emb_d = nc.dram_tensor("emb", [P, S * D], bf, kind="ExternalInput")
    ctab_d = nc.dram_tensor("ctab", [P, K * 4 * D], bf, kind="ExternalInput")
    pacc_d = nc.dram_tensor("pacc", [P, 4], f32, kind="ExternalOutput")

    off = np.concatenate([[0], np.cumsum(C)])

    # tree chunks over the (k,t) slot range: (s0, s1, engine). POOL covers
    # the tail ~46% of slots (its per-elem tree rate is ~3.8x DVE's, but it
    # runs in parallel and DVE is loaded with the diff pass + finals).
    pool_frac = 0.46
    s_pool = int(S * (1 - pool_frac))
    dve_rngs, pool_rngs = [], []
    ndc, npc = 6, 4
    b1 = [s_pool * i // ndc for i in range(ndc + 1)]
    for i in range(ndc):
        if b1[i + 1] > b1[i]:
            dve_rngs.append((b1[i], b1[i + 1]))
    b2 = [s_pool + (S - s_pool) * i // npc for i in range(npc + 1)]
    for i in range(npc):
        if b2[i + 1] > b2[i]:
            pool_rngs.append((b2[i], b2[i + 1]))

    with TileContext(nc) as tc:
        with tc.tile_pool(name="sbuf", bufs=1) as pool:
            emb = pool.tile([P, S, D], bf)
            ctab = pool.tile([P, K, 4, D], bf)
            dt_ = pool.tile([P, S4, D], bf)
            l1 = pool.tile([P, S4, 8], bf)
            l2 = pool.tile([P, S4, 4], bf)
            l3 = pool.tile([P, S4, 2], bf)
            dist = pool.tile([P, S4], bf)
            sq = pool.tile([P, S, 4], bf)
            pacc = pool.tile([P, 4], f32)

            nc.sync.dma_start(
                out=ctab[:],
                in_=ctab_d.ap().rearrange("p (k b d) -> p k b d", b=4, d=D),
            )
            NCH = 4
            bnd = [S * i // NCH for i in range(NCH + 1)]
            for i in range(NCH):
                nc.sync.dma_start(
                    out=emb[:, bnd[i] : bnd[i + 1], :],
                    in_=emb_d.ap().rearrange("p (t d) -> p t d", d=D)[
                        :, bnd[i] : bnd[i + 1], :
                    ],
                )

            # diff: dt[p, (k,t), b, d] = emb[p,(k,t),d] - cents[b,k,d]  (2x)
            for k in range(K):
                o0, o1 = int(off[k]), int(off[k + 1])
                ck = o1 - o0
                nc.vector.tensor_tensor(
                    out=dt_[:, o0 * 4 : o1 * 4, :].rearrange(
                        "p (t b) d -> p t b d", b=4
                    ),
                    in0=emb[:, o0:o1, :].unsqueeze(2).broadcast_to([P, ck, 4, D]),
                    in1=ctab[:, k, :, :].unsqueeze(1).broadcast_to([P, ck, 4, D]),
                    op=mybir.AluOpType.subtract,
                )

            # |.| in-place on ACT (memzero-style in-place activation)
            NAB = 8
            ab = [S4 * i // NAB for i in range(NAB + 1)]
            for i in range(NAB):
                ap = dt_[:, ab[i] : ab[i + 1], :].rearrange("p a b -> p (a b)")
                nc.scalar.activation(
                    out=ap, in_=ap, func=mybir.ActivationFunctionType.Abs
                )

            # Manhattan sum over d: log2 tree of TT adds (DVE 2x / POOL)
            with nc.allow_low_precision("bf16 L1 tree; error averages out"):
                for rngs, eng in ((dve_rngs, nc.vector), (pool_rngs, nc.gpsimd)):
                    for s0, s1 in rngs:
                        r = slice(s0 * 4, s1 * 4)
                        eng.tensor_tensor(
                            out=l1[:, r, :], in0=dt_[:, r, 0:8],
                            in1=dt_[:, r, 8:16], op=mybir.AluOpType.add)
                        eng.tensor_tensor(
                            out=l2[:, r, :], in0=l1[:, r, 0:4],
                            in1=l1[:, r, 4:8], op=mybir.AluOpType.add)
                        eng.tensor_tensor(
                            out=l3[:, r, :], in0=l2[:, r, 0:2],
                            in1=l2[:, r, 2:4], op=mybir.AluOpType.add)
                        eng.tensor_tensor(
                            out=dist[:, r].unsqueeze(2), in0=l3[:, r, 0:1],
                            in1=l3[:, r, 1:2], op=mybir.AluOpType.add)

                # pacc[p, b] = sum_t dist^2
                dview = dist[:].rearrange("p (t b) -> p t b", b=4)
                nc.vector.tensor_tensor(
                    out=sq[:], in0=dview, in1=dview, op=mybir.AluOpType.mult
                )
                nc.vector.tensor_reduce(
                    out=pacc[:],
                    in_=sq[:].rearrange("p t b -> p b t"),
                    axis=mybir.AxisListType.X,
                    op=mybir.AluOpType.add,
                )
            nc.sync.dma_start(out=pacc_d.ap(), in_=pacc[:])
    nc.compile()
    return nc


def _get(kind, C, S):
    key = (kind, tuple(int(x) for x in C))
    if key not in _built:
        _built[key] = (_build_a if kind == "A" else _build_b)(C, S)
    return _built[key]


# ---------------------------------------------------------------- host math

def _cents_from_partials(lay, results):
    """A results -> cents [B, K, D] f64 (exact counts from the sort)."""
    sums = np.zeros((B, K, D), dtype=np.float64)
    for c in range(NCORES):
        p = results[c]["part"].astype(np.float64).reshape(P, K, D)
        sums[c // 2] += p.sum(axis=0)
    cnt = np.zeros((B, K), dtype=np.float64)
    for c in range(NCORES):
        cnt[c // 2] += lay["counts"][c]
    cents = np.where(cnt[:, :, None] > 0, sums / np.maximum(cnt, 1.0)[:, :, None], 0.0)
    return cents


def _push_host(cents):
    d = np.abs(cents[:, :, None, :] - cents[:, None, :, :]).sum(-1)  # [B,K,K]
    m = np.maximum(PUSH_MARGIN - d, 0.0)
    triu = np.triu(np.ones((K, K), dtype=bool), k=1)
    return (m * m * triu[None]).sum(axis=(1, 2)) / NCMP  # [B]


def _finish(lay, cents, resultsB):
    cbf = cents.astype(BF16).astype(np.float64)  # table actually used on-chip
    raw = np.zeros(4, dtype=np.float64)
    for c in range(NCORES):
        raw += resultsB[c]["pacc"].astype(np.float64).sum(axis=0)
    # zero-pad slots contribute n_pad_k * L1(c_bk)^2 per core
    l1c = np.abs(cbf).sum(-1)  # [B, K]
    npad = (lay["C"][None, :] * P - lay["counts"]).astype(np.float64)  # [core,K]
    corr = (npad.sum(axis=0)[None, :] * (l1c ** 2)).sum(axis=1)  # [B]
    pull = (raw - corr) / NPIX_TOT
    push = _push_host(cents)
    return np.array(np.mean(PUSH_W * push + PULL_W * pull), dtype=F32)


# ---------------------------------------------------------------- driver

def prep_all(embeddings, labels):
    emb_flat = np.ascontiguousarray(np.asarray(embeddings), dtype=F32).reshape(
        NPIX_TOT, D
    )
    lab_flat = np.ascontiguousarray(np.asarray(labels), dtype=np.int32).reshape(
        NPIX_TOT
    )
    lay = _layout(lab_flat)
    lay["emb_q"] = _emb_sorted(emb_flat, lay)
    in_a = [{"emb": e} for e in lay["emb_q"]]
    return lay, in_a


def prep_b(lay, cents):
    ctab = np.ascontiguousarray(
        np.broadcast_to(
            cents.transpose(1, 0, 2).astype(BF16).reshape(1, K * 4 * D),
            (P, K * 4 * D),
        )
    )
    return [{"emb": e, "ctab": ctab} for e in lay["emb_q"]]


def run_launches(embeddings, labels, trace=False, trace_kwargs=None):
    lay, in_a = prep_all(embeddings, labels)
    core_ids = list(range(NCORES))
    kw = dict(trace=trace, **(trace_kwargs or {}))
    ncA = _get("A", lay["C"], lay["S"])
    resA = run_bass_kernel_spmd(ncA, in_a, core_ids, **kw)
    cents = _cents_from_partials(lay, resA.results)
    ncB = _get("B", lay["C"], lay["S"])
    resB = run_bass_kernel_spmd(ncB, prep_b(lay, cents), core_ids, **kw)
    loss = _finish(lay, cents, resB.results)
    return loss, resA, resB


def kernel(embeddings, labels):
    loss, _, _ = run_launches(embeddings, labels, trace=False)
    return loss


# revision 5
# speedup vs baseline: 1.3703x; 1.0797x over previous
"""Trainium2 Bass kernel for nn_MetricLoss (segment_reduce / discriminative loss).

Reference math (K=32 labels, D=16):
  cents[s,k,:]  = mean of embeddings of sample s where label==k
  push[s]       = sum_{k<j} relu(0.25 - L1(c_sk, c_sj))^2 / 496
  pull[s]       = mean over ALL B*H*W pixels p of  L1(e_p, c_s,label_p)^2
  loss          = mean_s (push[s] + 0.1 * pull[s])

Strategy (8 cores, 2 launches, SORT-BASED):
  Host sorts each core's 73728 pixels by label into a balanced layout
  [128 partitions, S slots, 16], where label k owns a uniform column
  range of C_k slots on every partition (C_k = max over cores of
  ceil(count/128)); unused slots are zero-padded.  Pixel order is
  irrelevant because both the centroid sums and the pull term are plain
  sums over pixels.

  Launch A: per-core centroid partial sums = per-label-group
    TensorReduce over the (contiguous, host-transposed [P,D,S]) slot
    axis -> partials [128, 16*32] f32.  Host reduces partitions + core
    pairs, divides by exact counts -> cents; push computed on host.

  Launch B: pull distances.  ctab [128, 32,4,16] = bf16 cents
    replicated per partition.  Per label-group: diff = emb - c
    broadcast (DVE TT, 2x mode); |.| in-place on ACT (Abs); Manhattan
    sum over d via a log2 tree of TT adds, chunked DVE/POOL (POOL owns
    the earliest slot chunks so it starts first).  dist [128, S*4] bf16
    is DMA'd out per chunk; host squares, masks pads, and reduces.
"""

import numpy as np
import ml_dtypes

import concourse.bass as bass
import concourse.bacc as bacc
import concourse.mybir as mybir
from concourse.tile import TileContext
from concourse.bass_utils import run_bass_kernel_spmd

BF16 = ml_dtypes.bfloat16
F32 = np.float32

B, H, W, D, K = 4, 384, 384, 16, 32
NCORES = 8
NPIX_TOT = B * H * W              # 589824
NPIX = NPIX_TOT // NCORES         # 73728 per core
P = 128

PUSH_MARGIN = 0.25
PUSH_W = 1.0
PULL_W = 0.1
NCMP = K * (K - 1) / 2.0

_built = {}


# ---------------------------------------------------------------- layout

def _c_groups(C):
    """Runs of equal C in label order: list of (k0, k1, c)."""
    groups = []
    k0 = 0
    for k in range(1, K + 1):
        if k == K or C[k] != C[k0]:
            groups.append((k0, k, int(C[k0])))
            k0 = k
    return groups


def _layout(lab_flat):
    counts = np.zeros((NCORES, K), dtype=np.int64)
    idx_by = []
    for c in range(NCORES):
        lab = lab_flat[c * NPIX : (c + 1) * NPIX]
        counts[c] = np.bincount(lab, minlength=K)
        order = np.argsort(lab, kind="stable")
        idx_by.append(np.split(order, np.cumsum(counts[c])[:-1]))
    C = np.maximum(1, (counts.max(axis=0) + P - 1) // P).astype(np.int64)
    off = np.concatenate([[0], np.cumsum(C)])
    S = int(off[-1])
    pixmaps = []
    for c in range(NCORES):
        pm = np.full((P, S), -1, dtype=np.int64)
        for k in range(K):
            ck = int(counts[c, k])
            pad = np.full(int(C[k]) * P, -1, dtype=np.int64)
            pad[:ck] = idx_by[c][k] + c * NPIX
            pm[:, off[k] : off[k + 1]] = pad.reshape(int(C[k]), P).T
        pixmaps.append(pm)
    return {"C": C, "off": off, "S": S, "counts": counts, "pixmaps": pixmaps}


def _emb_sorted(emb_flat, lay):
    S = lay["S"]
    emb_pad = np.vstack([emb_flat, np.zeros((1, D), dtype=emb_flat.dtype)])
    eq, et = [], []
    for pm in lay["pixmaps"]:
        g = emb_pad[np.where(pm < 0, NPIX_TOT, pm)].astype(BF16)  # [P, S, D]
        eq.append(np.ascontiguousarray(g.reshape(P, S * D)))
        et.append(np.ascontiguousarray(g.transpose(0, 2, 1).reshape(P, D * S)))
    return eq, et


# ---------------------------------------------------------------- launch A

def _build_a(C, S):
    nc = bacc.Bacc("TRN2", target_bir_lowering=False, debug=False)
    bf = mybir.dt.bfloat16
    f32 = mybir.dt.float32

    embt_d = nc.dram_tensor("embt", [P, D * S], bf, kind="ExternalInput")
    part_d = nc.dram_tensor("part", [P, D * K], f32, kind="ExternalOutput")

    off = np.concatenate([[0], np.cumsum(C)])
    groups = _c_groups(C)
    with TileContext(nc) as tc:
        with tc.tile_pool(name="sbuf", bufs=1) as pool:
            embt = pool.tile([P, D, S], bf)
            partials = pool.tile([P, D, K], f32)

            NCH = 2
            bnd = [S * i // NCH for i in range(NCH + 1)]
            for i in range(NCH):
                nc.sync.dma_start(
                    out=embt[:, :, bnd[i] : bnd[i + 1]],
                    in_=embt_d.ap().rearrange("p (d t) -> p d t", t=S)[
                        :, :, bnd[i] : bnd[i + 1]
                    ],
                )
            for k0, k1, c in groups:
                nc.vector.tensor_reduce(
                    out=partials[:, :, k0:k1],
                    in_=embt[:, :, off[k0] : off[k1]].rearrange(
                        "p d (g t) -> p d g t", t=c
                    ),
                    axis=mybir.AxisListType.X,
                    op=mybir.AluOpType.add,
                )
            nc.sync.dma_start(
                out=part_d.ap(), in_=partials[:].rearrange("p a b -> p (a b)")
            )
    nc.compile()
    return nc


# ---------------------------------------------------------------- launch B

def _build_b(C, S):
    nc = bacc.Bacc("TRN2", target_bir_lowering=False, debug=False)
    bf = mybir.dt.bfloat16
    S4 = S * 4

    emb_d = nc.dram_tensor("emb", [P, S * D], bf, kind="ExternalInput")
    ctab_d = nc.dram_tensor("ctab", [P, K * 4 * D], bf, kind="ExternalInput")
    dist_d = nc.dram_tensor("dist", [P, S4], bf, kind="ExternalOutput")

    off = np.concatenate([[0], np.cumsum(C)])
    groups = _c_groups(C)

    # tree chunks (slot ranges): POOL owns the first ~40% (starts early
    # while DVE still runs diffs), DVE the rest.
    s_pool = int(S * 0.40)
    rngs = []
    npc, ndc = 4, 3
    b1 = [s_pool * i // npc for i in range(npc + 1)]
    for i in range(npc):
        rngs.append((b1[i], b1[i + 1], "pool"))
    b2 = [s_pool + (S - s_pool) * i // ndc for i in range(ndc + 1)]
    for i in range(ndc):
        rngs.append((b2[i], b2[i + 1], "dve"))

    with TileContext(nc) as tc:
        with tc.tile_pool(name="sbuf", bufs=1) as pool:
            emb = pool.tile([P, S, D], bf)
            ctab = pool.tile([P, K, 4, D], bf)
            dt_ = pool.tile([P, S4, D], bf)
            l1 = pool.tile([P, S4, 8], bf)
            l2 = pool.tile([P, S4, 4], bf)
            l3 = pool.tile([P, S4, 2], bf)
            dist = pool.tile([P, S4], bf)

            nc.sync.dma_start(
                out=ctab[:],
                in_=ctab_d.ap().rearrange("p (k b d) -> p k b d", b=4, d=D),
            )
            NCH = 4
            bnd = [S * i // NCH for i in range(NCH + 1)]
            for i in range(NCH):
                nc.sync.dma_start(
                    out=emb[:, bnd[i] : bnd[i + 1], :],
                    in_=emb_d.ap().rearrange("p (t d) -> p t d", d=D)[
                        :, bnd[i] : bnd[i + 1], :
                    ],
                )

            # diff (DVE, 2x): dt[p,(k,t),b,d] = emb[p,(k,t),d] - cents[b,k,d]
            for k0, k1, c in groups:
                g = k1 - k0
                o0, o1 = int(off[k0]), int(off[k1])
                nc.vector.tensor_tensor(
                    out=dt_[:, o0 * 4 : o1 * 4, :].rearrange(
                        "p (g t b) d -> p g t b d", b=4, t=c
                    ),
                    in0=emb[:, o0:o1, :]
                    .rearrange("p (g t) d -> p g t d", t=c)
                    .unsqueeze(3)
                    .broadcast_to([P, g, c, 4, D]),
                    in1=ctab[:, k0:k1, :, :]
                    .unsqueeze(2)
                    .broadcast_to([P, g, c, 4, D]),
                    op=mybir.AluOpType.subtract,
                )

            # |.| in-place on ACT, chunk-aligned with the tree chunks
            for s0, s1, _ in rngs:
                ap = dt_[:, s0 * 4 : s1 * 4, :].rearrange("p a b -> p (a b)")
                nc.scalar.activation(
                    out=ap, in_=ap, func=mybir.ActivationFunctionType.Abs
                )

            # Manhattan sum over d: log2 tree of TT adds; dist out per chunk
            with nc.allow_low_precision("bf16 L1 tree; error averages out"):
                for s0, s1, eng_name in rngs:
                    eng = nc.gpsimd if eng_name == "pool" else nc.vector
                    r = slice(s0 * 4, s1 * 4)
                    eng.tensor_tensor(
                        out=l1[:, r, :], in0=dt_[:, r, 0:8],
                        in1=dt_[:, r, 8:16], op=mybir.AluOpType.add)
                    eng.tensor_tensor(
                        out=l2[:, r, :], in0=l1[:, r, 0:4],
                        in1=l1[:, r, 4:8], op=mybir.AluOpType.add)
                    eng.tensor_tensor(
                        out=l3[:, r, :], in0=l2[:, r, 0:2],
                        in1=l2[:, r, 2:4], op=mybir.AluOpType.add)
                    eng.tensor_tensor(
                        out=dist[:, r].unsqueeze(2), in0=l3[:, r, 0:1],
                        in1=l3[:, r, 1:2], op=mybir.AluOpType.add)
                    nc.sync.dma_start(out=dist_d.ap()[:, r], in_=dist[:, r])
    nc.compile()
    return nc


def _get(kind, C, S):
    key = (kind, tuple(int(x) for x in C))
    if key not in _built:
        _built[key] = (_build_a if kind == "A" else _build_b)(C, S)
    return _built[key]


# ---------------------------------------------------------------- host math

def _cents_from_partials(lay, results):
    sums = np.zeros((B, D, K), dtype=np.float64)
    for c in range(NCORES):
        p = results[c]["part"].astype(np.float64).reshape(P, D, K)
        sums[c // 2] += p.sum(axis=0)
    sums = sums.transpose(0, 2, 1)  # [B, K, D]
    cnt = np.zeros((B, K), dtype=np.float64)
    for c in range(NCORES):
        cnt[c // 2] += lay["counts"][c]
    return np.where(cnt[:, :, None] > 0, sums / np.maximum(cnt, 1.0)[:, :, None], 0.0)


def _push_host(cents):
    d = np.abs(cents[:, :, None, :] - cents[:, None, :, :]).sum(-1)  # [B,K,K]
    m = np.maximum(PUSH_MARGIN - d, 0.0)
    triu = np.triu(np.ones((K, K), dtype=bool), k=1)
    return (m * m * triu[None]).sum(axis=(1, 2)) / NCMP  # [B]


def _finish(lay, cents, resultsB):
    raw = np.zeros(4, dtype=np.float64)
    for c in range(NCORES):
        dist = resultsB[c]["dist"].astype(np.float32).reshape(P, lay["S"], 4)
        valid = (lay["pixmaps"][c] >= 0).astype(np.float32)  # [P, S]
        raw += ((dist * dist) * valid[:, :, None]).sum(axis=(0, 1)).astype(np.float64)
    pull = raw / NPIX_TOT
    push = _push_host(cents)
    return np.array(np.mean(PUSH_W * push + PULL_W * pull), dtype=F32)


# ---------------------------------------------------------------- driver

def prep_all(embeddings, labels):
    emb_flat = np.ascontiguousarray(np.asarray(embeddings), dtype=F32).reshape(
        NPIX_TOT, D
    )
    lab_flat = np.ascontiguousarray(np.asarray(labels), dtype=np.int32).reshape(
        NPIX_TOT
    )
    lay = _layout(lab_flat)
    lay["emb_q"], embt = _emb_sorted(emb_flat, lay)
    in_a = [{"embt": e} for e in embt]
    return lay, in_a


def prep_b(lay, cents):
    ctab = np.ascontiguousarray(
        np.broadcast_to(
            cents.transpose(1, 0, 2).astype(BF16).reshape(1, K * 4 * D),
            (P, K * 4 * D),
        )
    )
    return [{"emb": e, "ctab": ctab} for e in lay["emb_q"]]


def run_launches(embeddings, labels, trace=False, trace_kwargs=None):
    lay, in_a = prep_all(embeddings, labels)
    core_ids = list(range(NCORES))
    kw = dict(trace=trace, **(trace_kwargs or {}))
    ncA = _get("A", lay["C"], lay["S"])
    resA = run_bass_kernel_spmd(ncA, in_a, core_ids, **kw)
    cents = _cents_from_partials(lay, resA.results)
    ncB = _get("B", lay["C"], lay["S"])
    resB = run_bass_kernel_spmd(ncB, prep_b(lay, cents), core_ids, **kw)
    loss = _finish(lay, cents, resB.results)
    return loss, resA, resB


def kernel(embeddings, labels):
    loss, _, _ = run_launches(embeddings, labels, trace=False)
    return loss
